# revision 6
# baseline (speedup 1.0000x reference)
"""E90 dual-rate gated linear-attention scan on 8 Trainium2 cores.

Strategy
--------
Shard batch B=8 -> one batch element per NeuronCore (no cross-core comms).

The reference scan is, per (b, h), with per-timestep *scalar* decays:
    S_f[t] = df[t] * S_f[t-1] + kf[t] vf[t]^T          (fast, [KF,VF])
    S_s[t] = d's[t] * S_s[t-1] + g[t] ks[t] vs[t]^T     (slow, [KS,VS])
      where d's[t] = g[t]*ds[t] + (1-g[t])
    out[t] = mf[t] * qf[t] @ S_f[t] + ms[t] * qs[t] @ S_s[t]

Because decays are scalars we use the chunked linear-attention form with
chunk C=128.  Within a chunk let A[t] = prod_{j<=t} d[j] (inclusive local
cumprod).  All per-timestep scalars fold into host-side Q/K scalings:
    Qf~[t] = qf[t] * A_f[t] * mf[t]        Kf~[i] = kf[i] / A_f[i]
    Qs~[t] = qs[t] * A_s[t] * ms[t]        Ks~[i] = ks[i] * g[i] / A_s[i]
Then with causal mask M (valid i<=t), computed transposed (out^T[v,t]):
    out^T = S_f0^T Qf~^T + V_f^T masked(Kf~ Qf~^T) + (slow terms)
    S_f_new = A_fC * ( S_f0 + sum_i Kf~[i] vf[i]^T )   (+ slow)
The four output contributions accumulate in one PSUM tile via 4 matmuls
(lhsT = V / S natural layouts -> M=64 stationary loads).  The state update
accumulates K~^T V on top of an identity-matmul "keep" of the old state,
then one vector multiply by the per-chunk scalar A_C.

Fast-branch PE packing: fast per-head data is only 32 wide, so fast K/Q
transposes and fast states live in 4-way partition-stacked layouts
(head h -> row-group q=h%4, column-block g=h//4).  Fast score / inter /
state-update matmuls target distinct 32-row/col PE groups via
tile_position, so quads of heads execute concurrently in the array.
"""

import numpy as np

import concourse.bacc as bacc
import concourse.bass as bass
import concourse.mybir as mybir
import concourse.tile as tile
from concourse.bass_utils import run_bass_kernel_spmd
from concourse.masks import make_identity, make_upper_triangular

T, B, H = 2048, 8, 16
KF, VF, KS, VS = 32, 64, 128, 64
C = 128
NCH = T // C
NCORES = 8
F32 = mybir.dt.float32


def _offsets(n_heads):
    off_kslt = 0
    off_qslt = off_kslt + n_heads * C
    off_ksln = off_qslt + n_heads * C
    off_kfn = off_ksln + n_heads * KS
    off_vf = off_kfn + n_heads * KF
    off_vsl = off_vf + n_heads * VF
    off_kfqf = off_vsl + n_heads * VS
    x128 = off_kfqf + (n_heads // 4) * 2 * C  # 4-way stacked kfT|qfT
    return off_kslt, off_qslt, off_ksln, off_kfn, off_vf, off_vsl, off_kfqf, x128


def build_nc(t_len=T, n_heads=H):
    nch = t_len // C
    nH = n_heads
    assert nH % 4 == 0
    ng = nH // 4  # column-blocks in the 4-way fast layouts
    o_kslt, o_qslt, o_ksln, o_kfn, o_vf, o_vsl, o_kfqf, x128 = _offsets(nH)

    nc = bacc.Bacc()
    in128 = nc.dram_tensor("in128", [nch, 128, x128], F32, kind="ExternalInput")
    afc_d = nc.dram_tensor("afc", [128, nch * 4], F32, kind="ExternalInput")
    asc_d = nc.dram_tensor("asc", [128, nch * nH], F32, kind="ExternalInput")
    sf0_d = nc.dram_tensor("sf0", [128, ng * VF], F32, kind="ExternalInput")
    ss0_d = nc.dram_tensor("ss0", [KS, nH * VS], F32, kind="ExternalInput")
    out_d = nc.dram_tensor("out", [nch, VF, nH, C], F32, kind="ExternalOutput")
    sf_d = nc.dram_tensor("sf_out", [128, ng * VF], F32, kind="ExternalOutput")
    ss_d = nc.dram_tensor("ss_out", [KS, nH * VS], F32, kind="ExternalOutput")

    with tile.TileContext(nc) as tc:
        with (
            tc.tile_pool(name="const", bufs=1) as constp,
            tc.tile_pool(name="state", bufs=1) as statep,
            tc.tile_pool(name="ld128", bufs=2) as ld128p,
            tc.tile_pool(name="msk", bufs=2) as mskp,
            tc.tile_pool(name="stage", bufs=2) as stagep,
            tc.tile_pool(name="ps_sc", bufs=1, space="PSUM") as ps_sc,
            tc.tile_pool(name="ps_out", bufs=2, space="PSUM") as ps_out,
            tc.tile_pool(name="ps_st", bufs=1, space="PSUM") as ps_st,
        ):
            # constants
            mask4 = constp.tile([C, 4 * C], F32, tag="mask4")
            for k in range(4):
                make_upper_triangular(nc, mask4[:, k * C : (k + 1) * C], val=1.0)
            ident = constp.tile([KS, KS], F32, tag="ident")
            make_identity(nc, ident[:])
            afc_t = constp.tile([128, nch * 4], F32, tag="afc")
            asc_t = constp.tile([128, nch * nH], F32, tag="asc")
            nc.sync.dma_start(afc_t[:], afc_d[:])
            nc.sync.dma_start(asc_t[:], asc_d[:])

            # persistent states: Sf 4-way partition-stacked, Ss head-blocked
            Sf = statep.tile([128, ng * VF], F32, tag="Sf")
            Ss = statep.tile([KS, nH * VS], F32, tag="Ss")
            nc.sync.dma_start(Sf[:], sf0_d[:])
            nc.sync.dma_start(Ss[:], ss0_d[:])

            n_sstile = (nH + 7) // 8  # slow-state psum tiles of up to 8 heads

            for n in range(nch):
                sl = ld128p.tile([128, x128], F32, tag="sl")
                nc.sync.dma_start(sl[:], in128[n])
                stg = stagep.tile([VF, nH * C], F32, tag="stg")

                # --- state psum tiles + wide identity keeps ---
                sfp = ps_st.tile([128, ng * VF], F32, tag="sfp")
                for q in range(4):
                    nc.tensor.matmul(
                        sfp[32 * q : 32 * (q + 1), :],
                        ident[32 * q : 32 * (q + 1), 32 * q : 32 * (q + 1)],
                        Sf[32 * q : 32 * (q + 1), :],
                        start=True, stop=False,
                        tile_position=(32 * q, 32 * q),
                        skip_group_check=True,
                    )
                ssp = []
                for i in range(n_sstile):
                    hh = min(8, nH - 8 * i)
                    s = ps_st.tile([KS, hh * VS], F32, name=f"ssp{i}", tag=f"ssp{i}")
                    ssp.append(s)
                    nc.tensor.matmul(
                        s[:], ident[:],
                        Ss[:, 8 * i * VS : (8 * i + hh) * VS],
                        start=True, stop=False,
                        skip_group_check=True,
                    )

                for g in range(ng):  # quad of heads 4g..4g+3
                    scf = ps_sc.tile([C, 4 * C], F32, tag="scf")
                    scs = ps_sc.tile([C, 4 * C], F32, tag="scs")
                    for q in range(4):
                        h = 4 * g + q
                        kq0 = o_kfqf + g * 2 * C
                        # fast scores: distinct row-groups -> concurrent
                        nc.tensor.matmul(
                            scf[:, q * C : (q + 1) * C],
                            sl[32 * q : 32 * (q + 1), kq0 : kq0 + C],
                            sl[32 * q : 32 * (q + 1), kq0 + C : kq0 + 2 * C],
                            start=True, stop=True,
                            tile_position=(32 * q, 0),
                        )
                        nc.tensor.matmul(
                            scs[:, q * C : (q + 1) * C],
                            sl[:, o_kslt + h * C : o_kslt + (h + 1) * C],
                            sl[:, o_qslt + h * C : o_qslt + (h + 1) * C],
                            start=True, stop=True,
                        )
                    mskf = mskp.tile([C, 4 * C], F32, tag="mskf")
                    msks = mskp.tile([C, 4 * C], F32, tag="msks")
                    nc.vector.tensor_mul(mskf[:], scf[:], mask4[:])
                    nc.vector.tensor_mul(msks[:], scs[:], mask4[:])

                    # --- outputs, transposed: out^T[v, t] ---
                    op = ps_out.tile([VF, 4 * C], F32, tag="op")
                    for q in range(4):
                        h = 4 * g + q
                        o = op[:, q * C : (q + 1) * C]
                        kq0 = o_kfqf + g * 2 * C
                        nc.tensor.matmul(
                            o,
                            Sf[32 * q : 32 * (q + 1), g * VF : (g + 1) * VF],
                            sl[32 * q : 32 * (q + 1), kq0 + C : kq0 + 2 * C],
                            start=True, stop=False,
                            tile_position=(32 * q, 0),
                        )
                        nc.tensor.matmul(
                            o,
                            sl[:, o_vf + h * VF : o_vf + (h + 1) * VF],
                            mskf[:, q * C : (q + 1) * C],
                            start=False, stop=False,
                        )
                        nc.tensor.matmul(
                            o,
                            Ss[:, h * VS : (h + 1) * VS],
                            sl[:, o_qslt + h * C : o_qslt + (h + 1) * C],
                            start=False, stop=False,
                        )
                        nc.tensor.matmul(
                            o,
                            sl[:, o_vsl + h * VS : o_vsl + (h + 1) * VS],
                            msks[:, q * C : (q + 1) * C],
                            start=False, stop=True,
                        )

                        # --- state updates: accumulate K~^T V ---
                        nc.tensor.matmul(
                            sfp[32 * q : 32 * (q + 1), g * VF : (g + 1) * VF],
                            sl[:, o_kfn + h * KF : o_kfn + (h + 1) * KF],
                            sl[:, o_vf + h * VF : o_vf + (h + 1) * VF],
                            start=False, stop=(g == ng - 1),
                            tile_position=(0, 32 * q),
                            skip_group_check=True,
                        )
                        i, r = h // 8, h % 8
                        hh = min(8, nH - 8 * i)
                        nc.tensor.matmul(
                            ssp[i][:, r * VS : (r + 1) * VS],
                            sl[:, o_ksln + h * KS : o_ksln + (h + 1) * KS],
                            sl[:, o_vsl + h * VS : o_vsl + (h + 1) * VS],
                            start=False, stop=(r == hh - 1),
                            skip_group_check=True,
                        )

                    nc.scalar.copy(stg[:, 4 * g * C : 4 * (g + 1) * C], op[:])

                # --- scale states by per-chunk A_C ---
                afc_v = afc_t[:, n * 4 : n * 4 + ng][:, :, None].broadcast_to(
                    [128, ng, VF]
                )
                nc.vector.tensor_mul(
                    Sf[:].rearrange("p (g v) -> p g v", v=VF),
                    sfp[:].rearrange("p (g v) -> p g v", v=VF),
                    afc_v,
                )
                for i in range(n_sstile):
                    hh = min(8, nH - 8 * i)
                    col0 = n * nH + 8 * i
                    asc_v = asc_t[0:KS, col0 : col0 + hh][:, :, None].broadcast_to(
                        [KS, hh, VS]
                    )
                    nc.vector.tensor_mul(
                        Ss[:, 8 * i * VS : (8 * i + hh) * VS].rearrange(
                            "p (h v) -> p h v", v=VS
                        ),
                        ssp[i][:].rearrange("p (h v) -> p h v", v=VS),
                        asc_v,
                    )

                nc.sync.dma_start(out_d[n], stg[:])

            nc.sync.dma_start(sf_d[:], Sf[:])
            nc.sync.dma_start(ss_d[:], Ss[:])

    nc.compile()
    return nc


def _host_pack(inputs, b, t_len=T, n_heads=H):
    """Build the per-core input arrays (all scalar folding happens here)."""
    nch = t_len // C
    nH = n_heads
    ng = nH // 4
    o_kslt, o_qslt, o_ksln, o_kfn, o_vf, o_vsl, o_kfqf, x128 = _offsets(nH)

    def chunked(x):  # [T,H,...] -> [NCH,C,H,...] float64
        return x[:t_len, b, :nH].astype(np.float64).reshape(nch, C, nH, -1)

    df = chunked(inputs["decay_fast"])[..., 0]
    g = chunked(inputs["slow_gate"])[..., 0]
    ds = chunked(inputs["decay_slow"])[..., 0]
    mf = chunked(inputs["mix_fast"])[..., 0]
    ms = chunked(inputs["mix_slow"])[..., 0]
    Af = np.cumprod(df, axis=1)
    As = np.cumprod(g * ds + (1.0 - g), axis=1)

    qf_s = (chunked(inputs["q_fast"]) * (Af * mf)[..., None]).astype(np.float32)
    kf_s = (chunked(inputs["k_fast"]) / Af[..., None]).astype(np.float32)
    qs_s = (chunked(inputs["q_slow"]) * (As * ms)[..., None]).astype(np.float32)
    ks_s = (chunked(inputs["k_slow"]) * (g / As)[..., None]).astype(np.float32)
    vf = chunked(inputs["v_fast"]).astype(np.float32)
    vs = chunked(inputs["v_slow"]).astype(np.float32)

    def t_pack(x):  # [n,c,h,d] -> [n, d, h*c]
        return np.ascontiguousarray(x.transpose(0, 3, 2, 1)).reshape(
            nch, x.shape[3], nH * C
        )

    def n_pack(x):  # [n,c,h,d] -> [n, c, h*d]
        return x.reshape(nch, C, -1)

    a = np.empty((nch, 128, x128), np.float32)
    a[:, :, o_kslt:o_qslt] = t_pack(ks_s)
    a[:, :, o_qslt:o_ksln] = t_pack(qs_s)
    a[:, :, o_ksln:o_kfn] = n_pack(ks_s)
    a[:, :, o_kfn:o_vf] = n_pack(kf_s)
    a[:, :, o_vf:o_vsl] = n_pack(vf)
    a[:, :, o_vsl:o_kfqf] = n_pack(vs)
    # 4-way stacked fast kT/qT: head h -> rows 32*(h%4), col block h//4
    for h in range(nH):
        q, gg = h % 4, h // 4
        base = o_kfqf + gg * 2 * C
        a[:, 32 * q : 32 * (q + 1), base : base + C] = kf_s[:, :, h, :].transpose(
            0, 2, 1
        )
        a[:, 32 * q : 32 * (q + 1), base + C : base + 2 * C] = qf_s[
            :, :, h, :
        ].transpose(0, 2, 1)

    # afc_pb[p, n*4+g] = AfC[n, 4g + p//32]
    afc_pb = np.zeros((128, nch * 4), np.float32)
    for p_grp in range(4):
        for gg in range(ng):
            afc_pb[32 * p_grp : 32 * (p_grp + 1), np.arange(nch) * 4 + gg] = Af[
                :, -1, 4 * gg + p_grp
            ].astype(np.float32)[None, :]

    asc = np.broadcast_to(
        As[:, -1, :].reshape(1, nch * nH), (128, nch * nH)
    ).astype(np.float32)

    sf0 = np.zeros((128, ng * VF), np.float32)
    for h in range(nH):
        q, gg = h % 4, h // 4
        sf0[32 * q : 32 * (q + 1), gg * VF : (gg + 1) * VF] = inputs["S_fast0"][
            b, h
        ].astype(np.float32)
    ss0 = (
        np.ascontiguousarray(inputs["S_slow0"][b, :nH].transpose(1, 0, 2))
        .reshape(KS, nH * VS)
        .astype(np.float32)
    )

    return {"in128": a, "afc": afc_pb, "asc": asc, "sf0": sf0, "ss0": ss0}


def _unpack_core(res, t_len=T, n_heads=H):
    nch = t_len // C
    nH = n_heads
    ng = nH // 4
    # out_d [nch, VF, nH, C] -> [(n c), h, v]
    out = np.ascontiguousarray(
        res["out"].astype(np.float32).transpose(0, 3, 2, 1)
    ).reshape(nch * C, nH, VF)
    sf = np.empty((nH, KF, VF), np.float32)
    for h in range(nH):
        q, gg = h % 4, h // 4
        sf[h] = res["sf_out"][32 * q : 32 * (q + 1), gg * VF : (gg + 1) * VF]
    ss = np.ascontiguousarray(
        res["ss_out"].astype(np.float32).reshape(KS, nH, VS).transpose(1, 0, 2)
    )
    return sf, ss, out


_NC_CACHE = None


def _get_nc():
    global _NC_CACHE
    if _NC_CACHE is None:
        _NC_CACHE = build_nc()
    return _NC_CACHE


def _run(inputs, trace=False):
    nc = _get_nc()
    in_maps = [_host_pack(inputs, b) for b in range(B)]
    r = run_bass_kernel_spmd(nc, in_maps, list(range(NCORES)), trace=trace)
    sf_final = np.empty((B, H, KF, VF), np.float32)
    ss_final = np.empty((B, H, KS, VS), np.float32)
    output = np.empty((T, B, H, VF), np.float32)
    for b in range(B):
        sf, ss, out = _unpack_core(r.results[b])
        sf_final[b] = sf
        ss_final[b] = ss
        output[:, b] = out
    return (sf_final, ss_final, output), r


def kernel(**inputs):
    outs, _ = _run(inputs, trace=False)
    return outs


def _install_trace_shims():
    """Dev-only: register the NTFF profile hook that the agent image's
    antenv lacks, and stub the artifact upload (no bucket creds here)."""
    import contextlib
    import ctypes
    import sys
    import types

    import concourse.bass_utils as bu

    bu.upload_artifacts = lambda tmpdir: f"local://{tmpdir}"

    name = "antenv.axon_hooks"
    if name in sys.modules:
        return
    lib = ctypes.CDLL("/opt/axon/libaxon_pjrt.so")
    if not hasattr(lib, "axon_start_nrt_profile"):
        return
    lib.axon_start_nrt_profile.argtypes = [
        ctypes.POINTER(ctypes.c_int64),
        ctypes.c_size_t,
    ]
    lib.axon_start_nrt_profile.restype = ctypes.c_int64
    lib.axon_stop_nrt_profile.argtypes = [ctypes.c_char_p]
    lib.axon_stop_nrt_profile.restype = ctypes.c_int64

    @contextlib.contextmanager
    def _hook(output_dir, device_ids):
        import jax

        jax.devices()
        if device_ids:
            ids = (ctypes.c_int64 * len(device_ids))(*device_ids)
            rc = lib.axon_start_nrt_profile(ids, len(device_ids))
        else:
            rc = lib.axon_start_nrt_profile(None, 0)
        if rc != 0:
            raise RuntimeError(f"axon_start_nrt_profile rc={rc}")
        try:
            yield
        finally:
            n = lib.axon_stop_nrt_profile(str(output_dir).encode())
            print(f"profile: {n} file(s) written to {output_dir}")

    mod = types.ModuleType(name)
    mod.get_axon_ntff_profile_hook = lambda: _hook
    sys.modules[name] = mod


def kernel_traced(**inputs):
    _install_trace_shims()
    outs, r = _run(inputs, trace=True)
    return outs, r
